# revision 3
# baseline (speedup 1.0000x reference)
"""Trainium2 kernel for nn_CODABlocks2D: full CODA block on-device in Fourier space.

Sharding: heads 4c..4c+3 on core c for K/Q/V/attention/proj (x replicated via
on-device AllGather); final per-token stages token-sharded 8/core via an
on-device ReduceScatter of projection partials.  Attention scores are computed
directly in truncated-mode space (Parseval); K/Q/V images are never
materialized.  I/O per core: x shard (0.5MB), const tables (~0.6MB), output
shard (0.5MB).

Mode layout: XfS/XnS [128 par, 2112]: partitions 0..63 = Re over m-rows
(SEL64), 64..127 = Im; free = (t major 64, n minor 33).
Token-major: XnP_tok [64 t, (n 32 major, m 64 minor)].
"""
import sys
import time

import numpy as np

sys.path.insert(0, "/opt/trn_rl_repo")

EPS = 1e-5
NH = 32
SEL64 = np.r_[0:32, 96:128]
NCOL = 33
PERM = np.r_[0, 64 - np.arange(1, 64)]

LAST_EXEC_NS = None
_NC = None


# ---------------------------------------------------------------------------
# Host constant tables
# ---------------------------------------------------------------------------

def _dft_mats():
    h = np.arange(128)
    FrT = np.exp(-2j*np.pi*np.outer(h, SEL64)/128) / (128.0*128.0)   # [128h,64m]
    G = np.exp(-2j*np.pi*np.outer(h, np.arange(NCOL))/128)           # [128w,33n]
    P = np.exp(2j*np.pi*np.outer(h, SEL64)/128)                      # [128h,64m]
    gam = np.ones(NCOL); gam[1:32] = 2.0
    Q = gam[:, None]*np.exp(2j*np.pi*np.outer(np.arange(NCOL), h)/128)
    Q[0, :] = Q[0, :].real
    return FrT, G, P, Q


def _conv_pad(w):
    wc = w[..., 0] + 1j*w[..., 1]
    C = np.zeros((NH, 64, NCOL), np.complex128)
    C[:, 0:16, 0:16] = wc[0, 0]
    C[:, 48:64, 0:16] = wc[1, 0]
    return C


def _make_consts(args):
    FrT, G, P, Q = _dft_mats()
    f32 = np.float32
    c = {}
    c["FrT2"] = np.ascontiguousarray(
        np.concatenate([FrT.real, FrT.imag], 1).astype(f32))
    c["G2"] = np.ascontiguousarray(
        np.concatenate([G.real, G.imag], 1).astype(f32))
    c["G2m"] = np.ascontiguousarray(
        np.concatenate([G.real[:, 0:32], G.imag[:, 0:32]], 1).astype(f32))
    c["IDENT"] = np.eye(128, dtype=f32)
    c["QINV"] = np.ascontiguousarray(np.concatenate(
        [Q[0:32].real, Q[0:32].imag, -Q[0:32].imag], 1).astype(f32))   # [32,384]
    c["PINV"] = np.ascontiguousarray(np.concatenate(
        [P.real.T, -P.imag.T], 1).astype(f32))                          # [64,256]
    Pp = np.zeros((128, 128), f32)
    for m in range(64):
        Pp[PERM[m], m] = 1.0
        Pp[64 + PERM[m], 64 + m] = 1.0
    c["PPERM"] = Pp

    CK = _conv_pad(args["wK"]) + args["wKs"][:, 0][:, None, None]
    CQ = _conv_pad(args["wQ"]) + args["wQs"][:, 0][:, None, None]
    CV = np.zeros((NH, 64, NCOL), np.complex128)
    wcV = args["wV"][..., 0] + 1j*args["wV"][..., 1]
    CV[:, 0:16, 0:16] = wcV[0, 0]
    CV[:, 48:64, 0:16] = wcV[1, 0]
    wcP = args["wP"][..., 0] + 1j*args["wP"][..., 1]
    WP = np.zeros((NH, 64, 32), np.complex128)
    WP[:, 0:32] = wcP[0, :, 0]
    WP[:, 32:64] = wcP[1, :, 0]

    alpha = np.full(NCOL, 128.0); alpha[0] = 32.0; alpha[32] = 32.0
    b0v = 32.0
    D = alpha[None, None, :]*CQ*np.conj(CK)
    D[:, 0, 0] = 0.0
    c["DK"] = np.stack([D.real, D.imag], 1).astype(f32)            # [NH,2,64,33]
    E0 = b0v*CQ[:, :, 0]*CK[:, PERM, 0]; E0[:, 0] = 0.0
    E32 = b0v*CQ[:, :, 32]*CK[:, PERM, 32]
    E = np.stack([E0, E32], -1)                                    # [NH,64,2]
    c["EP"] = np.stack([E.real, E.imag], 1).astype(f32)            # [NH,2,64,2]

    # DC ext: [XnRdc; XnIdc; 1] @ DCWM -> [qdR, qdI, kdR', kdI'] per head
    a0 = alpha[0]
    dcwm = np.zeros((NH, 3, 4), f32)
    dcwm[:, 0, 0] = CQ[:, 0, 0].real
    dcwm[:, 1, 0] = -CQ[:, 0, 0].imag
    dcwm[:, 2, 0] = args["bQs"]
    dcwm[:, 0, 1] = CQ[:, 0, 0].imag
    dcwm[:, 1, 1] = CQ[:, 0, 0].real
    dcwm[:, 0, 2] = (a0 + b0v)*CK[:, 0, 0].real
    dcwm[:, 1, 2] = -(a0 + b0v)*CK[:, 0, 0].imag
    dcwm[:, 2, 2] = (a0 + b0v)*args["bKs"]
    dcwm[:, 0, 3] = (a0 - b0v)*CK[:, 0, 0].imag
    dcwm[:, 1, 3] = (a0 - b0v)*CK[:, 0, 0].real
    c["DCWM"] = dcwm

    CVs = CV.copy()
    c0 = CVs[:, :, 0].copy()
    n0 = 0.5*(c0 + np.conj(c0[:, PERM]))
    n0[:, 32] = 0.5*c0[:, 32]
    n0[:, 0] = c0[:, 0].real
    CVs[:, :, 0] = n0
    CVf = np.transpose(CVs[:, :, 0:32], (0, 2, 1))                 # [NH,32n,64m]
    c["CVF"] = np.ascontiguousarray(
        np.stack([CVf.real, CVf.imag], 1).reshape(NH, 2, 2048).astype(f32))
    WPf = np.transpose(WP, (0, 2, 1))
    c["WPF"] = np.ascontiguousarray(
        np.stack([WPf.real, WPf.imag], 1).reshape(NH, 2, 2048).astype(f32))

    def mixw(w):
        wc = w[..., 0] + 1j*w[..., 1]
        M = np.zeros((64, 32), np.complex128)
        M[0:32] = wc[0, 0, 0]
        M[32:64] = wc[1, 0, 0]
        return M.T
    CM0, CM1 = mixw(args["wM0"]), mixw(args["wM1"])
    c["CMIX"] = np.ascontiguousarray(np.concatenate(
        [CM0.real, CM0.imag, CM1.real, CM1.imag], 1).astype(f32))  # [32,256]
    c["ONES"] = np.ones((128, 1), f32)

    g, bb = args["norm_g"], args["norm_b"]
    const1 = float(args["bPs"][0] + np.dot(args["wPs"][0], args["bVs"]))
    crow = np.zeros((1, 64), f32)
    for k, v in {0: g[0], 1: bb[0], 2: g[1], 3: bb[1], 4: g[2], 5: bb[2],
                 6: g[3], 7: bb[3], 8: g[4], 9: bb[4], 10: g[5], 11: bb[5],
                 28: const1, 29: args["wM0s"][0, 0], 30: args["bM0s"][0],
                 31: args["wM1s"][0, 0], 32: args["bM1s"][0]}.items():
        crow[0, k] = v
    c["_crow_base"] = crow
    c["_W"] = (args["wVs"][:, 0].astype(f32), args["wPs"][0].astype(f32),
               args["bVs"].astype(f32))
    return c


# (name, shape) layout of the consolidated const tensor; per-core entries
# (DK/EP/DCWM/CROW/CVF/WPF) are appended by kernel() per core.
CT_TABLES = [("FrT2", (128, 128)), ("G2", (128, 66)), ("G2m", (128, 64)),
             ("IDENT", (128, 128)), ("QINV", (32, 384)), ("PINV", (64, 256)),
             ("PPERM", (128, 128)), ("CMIX", (32, 256)), ("ONES", (128, 1)),
             ("DK", (64, 264)), ("EP", (64, 16)), ("DCWM", (3, 16)),
             ("CROW", (1, 64)), ("CVF", (8, 2048)), ("WPF", (8, 2048))]
CT_OFF = {}
_o = 0
for _n, _sh in CT_TABLES:
    CT_OFF[_n] = _o
    _o += _sh[0]*_sh[1]
CT_TOT = _o


# ---------------------------------------------------------------------------
# Bass kernel
# ---------------------------------------------------------------------------

def _build_nc():
    import concourse.bacc as bacc
    import concourse.mybir as mybir
    from concourse.tile import TileContext
    from concourse.bass import AP
    from concourse.alu_op_type import AluOpType as alu

    f32 = mybir.dt.float32
    X = mybir.AxisListType.X
    AF = mybir.ActivationFunctionType
    GRP = [[0, 1, 2, 3, 4, 5, 6, 7]]
    REC = 8256

    nc = bacc.Bacc(None, target_bir_lowering=False)
    dt = nc.dram_tensor
    xs = dt("xs", [8, 16384], f32, kind="ExternalInput")
    CT = dt("CT", [1, CT_TOT], f32, kind="ExternalInput")

    xsi = dt("xsi", [8, 16384], f32, kind="Internal")
    xfull = dt("xfull", [64, 16384], f32, kind="Internal", addr_space="Shared")
    rsin = dt("rsin", [64, REC], f32, kind="Internal")
    rsout = dt("rsout", [8, REC], f32, kind="Internal")
    w2s = dt("w2s", [8, 16384], f32, kind="Internal")
    out = dt("out", [8, 16384], f32, kind="ExternalOutput")

    with TileContext(nc) as tc:
        with tc.tile_pool(name="const", bufs=1) as cpool, \
             tc.tile_pool(name="big", bufs=1) as bpool, \
             tc.tile_pool(name="wrk", bufs=2) as wpool, \
             tc.tile_pool(name="one", bufs=1) as opool, \
             tc.tile_pool(name="strip", bufs=2) as spool, \
             tc.tile_pool(name="ps_a", bufs=2, space="PSUM") as psa, \
             tc.tile_pool(name="ps_b", bufs=2, space="PSUM") as psb, \
             tc.tile_pool(name="ps_c", bufs=2, space="PSUM") as psc, \
             tc.tile_pool(name="ps_z", bufs=2, space="PSUM") as psz_pool:

            ctb = CT[0:1, :]

            def cload(name):
                rows, width = dict(CT_TABLES)[name]
                t = cpool.tile([rows, width], f32, tag=name)
                nc.sync.dma_start(
                    t, AP(ctb.tensor, CT_OFF[name], [[width, rows], [1, width]]))
                return t
            frt2 = cload("FrT2")
            g2 = cload("G2")
            g2m = cload("G2m")
            ident = cload("IDENT")
            qinv = cload("QINV")
            pinv = cload("PINV")
            pperm = cload("PPERM")
            dk = cload("DK")
            ept = cload("EP")
            dcwm = cload("DCWM")
            cmix = cload("CMIX")
            ones = cload("ONES")
            crow = cload("CROW")
            cbc = cpool.tile([128, 64], f32, tag="cbc")
            nc.gpsimd.partition_broadcast(cbc, crow[0:1, :])
            epst = cpool.tile([128, 1], f32, tag="epst")
            nc.vector.memset(epst, EPS)

            # ---- AllGather x (stage via SBUF chunks; collectives can't read IO) ----
            for ch in range(16):
                xg = opool.tile([8, 1024], f32, tag="xg")
                nc.sync.dma_start(xg, xs[:, 1024*ch:1024*ch+1024])
                nc.sync.dma_start(xsi[:, 1024*ch:1024*ch+1024], xg)
            nc.gpsimd.collective_compute(
                "AllGather", mybir.AluOpType.bypass, GRP,
                ins=[xsi[:, :]], outs=[xfull[:, :]])

            # ---- stats over all 64 tokens (chunked) ----
            s12 = cpool.tile([64, 2], f32, tag="s12")
            nc.vector.memset(s12, 0.0)
            for ch in range(8):
                xc = opool.tile([64, 2048], f32, tag="xc")
                nc.sync.dma_start(xc, xfull[:, 2048*ch:2048*ch+2048])
                pt = spool.tile([64, 2], f32, tag="pt")
                nc.vector.reduce_sum(pt[:, 0:1], xc, axis=X)
                nc.vector.tensor_tensor(xc, xc, xc, alu.mult)
                nc.vector.reduce_sum(pt[:, 1:2], xc, axis=X)
                nc.vector.tensor_tensor(s12, s12, pt, alu.add)
            stats = cpool.tile([64, 4], f32, tag="stats")  # mu, var, a, c
            nc.vector.tensor_scalar_mul(stats[:, 0:2], s12, 1.0/16384.0)
            musq = spool.tile([64, 1], f32, tag="musq")
            nc.vector.tensor_tensor(musq, stats[:, 0:1], stats[:, 0:1], alu.mult)
            nc.vector.tensor_tensor(stats[:, 1:2], stats[:, 1:2], musq, alu.subtract)
            rsv = spool.tile([64, 1], f32, tag="rsv")
            nc.scalar.activation(rsv, stats[:, 1:2], AF.Sqrt, bias=epst[0:64, 0:1])
            nc.vector.reciprocal(rsv, rsv)
            nc.vector.tensor_tensor(stats[:, 2:3], rsv, cbc[0:64, 0:1], alu.mult)
            nc.vector.tensor_tensor(musq, stats[:, 0:1], stats[:, 2:3], alu.mult)
            nc.vector.tensor_tensor(stats[:, 3:4], cbc[0:64, 1:2], musq, alu.subtract)
            ps_a1 = psa.tile([1, 64], f32, tag="A")
            nc.tensor.transpose(ps_a1, stats[:, 2:3], ident[0:64, 0:64])
            arow1 = cpool.tile([1, 64], f32, tag="arow1")
            nc.scalar.mul(arow1, ps_a1, 1.0)
            ps_c1 = psa.tile([1, 64], f32, tag="A")
            nc.tensor.transpose(ps_c1, stats[:, 3:4], ident[0:64, 0:64])
            crow2 = cpool.tile([1, 64], f32, tag="crow2")
            nc.scalar.mul(crow2, ps_c1, 1.0)
            # A_bc [128, 2112], layout (t, n): value a_t
            arep = cpool.tile([1, 2112], f32, tag="arep")
            sa_ = arow1[0:1, :]
            nc.vector.tensor_copy(
                arep.rearrange("p (t n) -> p t n", n=33),
                AP(sa_.tensor, sa_.offset, [sa_.ap[0], [1, 64], [0, 33]]))
            abc = cpool.tile([128, 2112], f32, tag="abc")
            nc.gpsimd.partition_broadcast(abc, arep[0:1, :])

            # ---- forward DFT per token -> xfs [128, 2112] (t,n layout) ----
            xfs = bpool.tile([128, 2112], f32, tag="xfs")
            for t in range(64):
                xtile = wpool.tile([128, 128], f32, tag="xtile")
                nc.sync.dma_start(
                    xtile, xfull[t].rearrange("(h w) -> h w", h=128))
                ps1 = psa.tile([128, 128], f32, tag="A")
                nc.tensor.matmul(ps1, xtile, frt2, start=True, stop=True)
                ct2 = wpool.tile([128, 128], f32, tag="ct2")
                nc.scalar.mul(ct2, ps1, 1.0)
                ps2 = psb.tile([128, 66], f32, tag="B")
                nc.tensor.matmul(ps2, ct2, g2, start=True, stop=True)
                sbA = wpool.tile([64, 66], f32, tag="sbA")
                sbB = wpool.tile([64, 66], f32, tag="sbB")
                nc.scalar.mul(sbA, ps2[0:64, :], 1.0)
                nc.scalar.mul(sbB, ps2[64:128, :], 1.0)
                nc.vector.tensor_tensor(xfs[0:64, 33*t:33*t+33],
                                        sbA[:, 0:33], sbB[:, 33:66],
                                        alu.subtract)
                nc.vector.tensor_tensor(xfs[64:128, 33*t:33*t+33],
                                        sbA[:, 33:66], sbB[:, 0:33],
                                        alu.add)

            # ---- normalize ----
            xns = bpool.tile([128, 2112], f32, tag="xns")
            nc.vector.tensor_tensor(xns, xfs, abc, alu.mult)
            bdc = xns[0:1, :]
            dcap = AP(bdc.tensor, bdc.offset, [bdc.ap[0], [33, 64]])
            nc.vector.tensor_tensor(dcap, dcap, crow2, alu.add)
            # base-0 copy of the Im half (same-base-partition rule for DVE)
            xnsI2 = abc[0:64, :]
            nc.vector.tensor_copy(xnsI2, xns[64:128, :])

            # ---- permuted strips (n=0, n=32) ----
            xnstr = cpool.tile([128, 128], f32, tag="xnstr")
            bx = xns[:, :]
            for j, n in ((0, 0), (1, 32)):
                nc.vector.tensor_copy(
                    xnstr[:, 64*j:64*j+64],
                    AP(bx.tensor, bx.offset + n, [bx.ap[0], [33, 64]]))
            ps_p = psa.tile([128, 128], f32, tag="A")
            nc.tensor.matmul(ps_p, pperm, xnstr, start=True, stop=True)
            xnps = cpool.tile([128, 128], f32, tag="xnps")
            nc.scalar.mul(xnps, ps_p, 1.0)
            xnpsI = cpool.tile([64, 128], f32, tag="xnpsI")
            nc.vector.tensor_copy(xnpsI, xnps[64:128, :])

            # ---- XnP_tok ----
            xnpR = bpool.tile([64, 2048], f32, tag="xnpR")
            xnpI = bpool.tile([64, 2048], f32, tag="xnpI")
            for n in range(32):
                ps_t = psc.tile([64, 128], f32, tag="C")
                nc.tensor.transpose(
                    ps_t,
                    AP(bx.tensor, bx.offset + n, [bx.ap[0], [33, 64]]),
                    ident)
                nc.scalar.mul(xnpR[:, 64*n:64*n+64], ps_t[:, 0:64], 1.0)
                nc.scalar.mul(xnpI[:, 64*n:64*n+64], ps_t[:, 64:128], 1.0)

            # ---- DC strips gather (partitions 0 and 64 -> [3, 64]) ----
            dcstr = spool.tile([3, 64], f32, tag="dcstr")
            nc.vector.memset(dcstr, 1.0)
            nc.sync.dma_start(
                dcstr[0:2, :],
                AP(bx.tensor, bx.offset, [[bx.ap[0][0]*64, 2], [33, 64]]))

            # ---- accumulators ----
            prjR = bpool.tile([64, 2048], f32, tag="prjR")
            prjI = bpool.tile([64, 2048], f32, tag="prjI")
            vskR = bpool.tile([64, 2048], f32, tag="vskR")
            vskI = bpool.tile([64, 2048], f32, tag="vskI")
            beta0 = cpool.tile([32, 32], f32, tag="beta0")
            beta1 = cpool.tile([32, 32], f32, tag="beta1")
            for tl in (prjR, prjI, vskR, vskI, beta0, beta1):
                nc.vector.memset(tl, 0.0)

            def tn(tile_ap, n, b):
                return AP(tile_ap.tensor, tile_ap.offset + 33*32*b + n,
                          [tile_ap.ap[0], [33, 32]])

            iw = {"wVs": 12, "wPs": 16, "wPwV": 20, "bVs": 24}
            attn_all = {}
            for o in range(4):
                # K-side main [128, 2112]; all products at base 0, then copy Im half
                ks = opool.tile([128, 2112], f32, tag="ks")
                v3 = lambda tl: tl.rearrange("p (t n) -> p t n", n=33)
                dkR = dk[:, 66*o:66*o+33]
                dkI = dk[:, 66*o+33:66*o+66]
                bc33 = lambda w: AP(w.tensor, w.offset, [w.ap[0], [0, 64], [1, 33]])
                xnR = xns[0:64, :]
                tmp0 = xfs[0:64, :]     # xfs is dead after normalization; scratch
                ksr = ks[0:64, :]
                # Im half first (built in ks[0:64], copied up)
                nc.vector.tensor_tensor(v3(ksr), v3(xnsI2), bc33(dkR), alu.mult)
                nc.vector.tensor_tensor(v3(tmp0), v3(xnR), bc33(dkI), alu.mult)
                nc.vector.tensor_tensor(ksr, ksr, tmp0, alu.subtract)
                nc.vector.tensor_copy(ks[64:128, :], ksr)
                # Re half
                nc.vector.tensor_tensor(v3(ksr), v3(xnR), bc33(dkR), alu.mult)
                nc.vector.tensor_tensor(v3(tmp0), v3(xnsI2), bc33(dkI), alu.mult)
                nc.vector.tensor_tensor(ksr, ksr, tmp0, alu.add)
                # K-perm [128, 128] layout (j, t)
                kp = wpool.tile([128, 128], f32, tag="kp")
                tpa = wpool.tile([64, 128], f32, tag="tpa")
                v2 = lambda tl: tl.rearrange("p (j t) -> p j t", t=64)
                epR = ept[:, 4*o:4*o+2]
                epI = ept[:, 4*o+2:4*o+4]
                bc2 = lambda w: AP(w.tensor, w.offset, [w.ap[0], [1, 2], [0, 64]])
                xpR = xnps[0:64, :]
                kpr = kp[0:64, :]
                # Im half = -(ER*XnpI + EI*XnpR), built low then copied up
                nc.vector.tensor_tensor(v2(kpr), v2(xnpsI), bc2(epR), alu.mult)
                nc.vector.tensor_tensor(v2(tpa), v2(xpR), bc2(epI), alu.mult)
                nc.vector.tensor_tensor(kpr, kpr, tpa, alu.add)
                nc.vector.tensor_scalar_mul(kpr, kpr, -1.0)
                nc.vector.tensor_copy(kp[64:128, :], kpr)
                # Re half = ER*XnpR - EI*XnpI
                nc.vector.tensor_tensor(v2(kpr), v2(xpR), bc2(epR), alu.mult)
                nc.vector.tensor_tensor(v2(tpa), v2(xnpsI), bc2(epI), alu.mult)
                nc.vector.tensor_tensor(kpr, kpr, tpa, alu.subtract)
                # DC ext [4, 64] = ([3,64]^T @ dcwm-slice)^T ... via matmul:
                ps_dq = psc.tile([2, 64], f32, tag="C")
                nc.tensor.matmul(ps_dq, dcwm[:, 4*o:4*o+2], dcstr, start=True, stop=True)
                qext = spool.tile([2, 64], f32, tag="qext")
                nc.vector.tensor_copy(qext, ps_dq)
                ps_dk = psc.tile([2, 64], f32, tag="C")
                nc.tensor.matmul(ps_dk, dcwm[:, 4*o+2:4*o+4], dcstr, start=True, stop=True)
                kext = spool.tile([2, 64], f32, tag="kext")
                nc.vector.tensor_copy(kext, ps_dk)
                for b in range(2):
                    ps_s = psb.tile([32, 32], f32, tag="B")
                    for n in range(33):
                        nc.tensor.matmul(ps_s, tn(bx, n, b), tn(ks[:, :], n, b),
                                         start=(n == 0), stop=False)
                    for j in range(2):
                        nc.tensor.matmul(ps_s, xnstr[:, 64*j+32*b:64*j+32*b+32],
                                         kp[:, 64*j+32*b:64*j+32*b+32],
                                         start=False, stop=False)
                    nc.tensor.matmul(ps_s, qext[:, 32*b:32*b+32],
                                     kext[:, 32*b:32*b+32], start=False, stop=True)
                    sc = spool.tile([32, 32], f32, tag="sc")
                    nc.scalar.mul(sc, ps_s, 1.0)
                    mx = spool.tile([32, 1], f32, tag="mx")
                    nc.vector.reduce_max(mx, sc, axis=X)
                    nmx = spool.tile([32, 1], f32, tag="nmx")
                    nc.vector.tensor_scalar_mul(nmx, mx, -1.0)
                    exv = spool.tile([32, 32], f32, tag="exv")
                    nc.scalar.activation(exv, sc, AF.Exp, bias=nmx[:, 0:1])
                    smv = spool.tile([32, 1], f32, tag="smv")
                    nc.vector.reduce_sum(smv, exv, axis=X)
                    rcv = spool.tile([32, 1], f32, tag="rcv")
                    nc.vector.reciprocal(rcv, smv)
                    at = spool.tile([32, 32], f32, tag=f"at{o}{b}")
                    nc.vector.tensor_scalar_mul(at, exv, rcv[:, 0:1])
                    ps_tt = psc.tile([32, 32], f32, tag="C")
                    nc.tensor.transpose(ps_tt, at, ident[0:32, 0:32])
                    att = spool.tile([32, 32], f32, tag=f"att{o}{b}")
                    nc.scalar.mul(att, ps_tt, 1.0)
                    attn_all[(o, b)] = (at, att)

            # ---- per-head Y build + mixes ----
            for o in range(4):
                cvR = opool.tile([64, 2048], f32, tag="cvR")
                cvI = opool.tile([64, 2048], f32, tag="cvI")
                wpR = opool.tile([64, 2048], f32, tag="wpR")
                wpI = opool.tile([64, 2048], f32, tag="wpI")
                for dst_, base_, row in ((cvR, CT_OFF["CVF"], 2*o),
                                         (cvI, CT_OFF["CVF"], 2*o+1),
                                         (wpR, CT_OFF["WPF"], 2*o),
                                         (wpI, CT_OFF["WPF"], 2*o+1)):
                    nc.sync.dma_start(
                        dst_, AP(ctb.tensor, base_ + 2048*row,
                                 [[0, 64], [1, 2048]]))
                for b in range(2):
                    rs_ = slice(32*b, 32*b+32)
                    wv = cbc[rs_, iw["wVs"]+o:iw["wVs"]+o+1]
                    bv = cbc[rs_, iw["bVs"]+o:iw["bVs"]+o+1]
                    wp_ = cbc[rs_, iw["wPs"]+o:iw["wPs"]+o+1]
                    wpwv = cbc[0:32, iw["wPwV"]+o:iw["wPwV"]+o+1]
                    at, att = attn_all[(o, b)]
                    bt_ = beta1 if b else beta0
                    nc.vector.scalar_tensor_tensor(
                        bt_, at, wpwv, bt_, alu.mult, alu.add)
                    for j in range(4):
                        cs = slice(512*j, 512*j+512)
                        xR = xnpR[rs_, cs]
                        xI = xnpI[rs_, cs]
                        yvR = opool.tile([64, 512], f32, tag="yvR")
                        yvI = opool.tile([64, 512], f32, tag="yvI")
                        tya = opool.tile([64, 512], f32, tag="tya")
                        yR = opool.tile([64, 512], f32, tag="yR")
                        yI = opool.tile([64, 512], f32, tag="yI")
                        nc.vector.tensor_tensor(yvR[rs_, :], xR, cvR[rs_, cs], alu.mult)
                        nc.vector.tensor_tensor(tya[rs_, :], xI, cvI[rs_, cs], alu.mult)
                        nc.vector.tensor_tensor(yvR[rs_, :], yvR[rs_, :], tya[rs_, :], alu.subtract)
                        nc.vector.tensor_tensor(yvI[rs_, :], xR, cvI[rs_, cs], alu.mult)
                        nc.vector.tensor_tensor(tya[rs_, :], xI, cvR[rs_, cs], alu.mult)
                        nc.vector.tensor_tensor(yvI[rs_, :], yvI[rs_, :], tya[rs_, :], alu.add)
                        nc.vector.scalar_tensor_tensor(
                            yR[rs_, :], xR, wv, yvR[rs_, :], alu.mult, alu.add)
                        nc.vector.scalar_tensor_tensor(
                            yI[rs_, :], xI, wv, yvI[rs_, :], alu.mult, alu.add)
                        if j == 0:
                            nc.vector.tensor_scalar_add(
                                yR[rs_, 0:1], yR[rs_, 0:1], bv)
                        if b == 0:
                            ryR, ryI = yR[0:32, :], yI[0:32, :]
                            ryvR, ryvI = yvR[0:32, :], yvI[0:32, :]
                        else:
                            ryR = opool.tile([32, 512], f32, tag="ryR")
                            ryI = opool.tile([32, 512], f32, tag="ryI")
                            ryvR = opool.tile([32, 512], f32, tag="ryvR")
                            ryvI = opool.tile([32, 512], f32, tag="ryvI")
                            nc.vector.tensor_copy(ryR, yR[32:64, :])
                            nc.vector.tensor_copy(ryI, yI[32:64, :])
                            nc.vector.tensor_copy(ryvR, yvR[32:64, :])
                            nc.vector.tensor_copy(ryvI, yvI[32:64, :])
                        tz = opool.tile([64, 512], f32, tag="tz")
                        tzs = tz[rs_, :]
                        psR = psz_pool.tile([32, 512], f32, tag="Z")
                        nc.tensor.matmul(psR, att, ryR, start=True, stop=True)
                        nc.vector.tensor_tensor(tzs, psR, wpR[rs_, cs], alu.mult)
                        nc.vector.tensor_tensor(prjR[rs_, cs], prjR[rs_, cs], tzs, alu.add)
                        nc.vector.tensor_tensor(tzs, psR, wpI[rs_, cs], alu.mult)
                        nc.vector.tensor_tensor(prjI[rs_, cs], prjI[rs_, cs], tzs, alu.add)
                        psI = psz_pool.tile([32, 512], f32, tag="Z")
                        nc.tensor.matmul(psI, att, ryI, start=True, stop=True)
                        nc.vector.tensor_tensor(tzs, psI, wpI[rs_, cs], alu.mult)
                        nc.vector.tensor_tensor(prjR[rs_, cs], prjR[rs_, cs], tzs, alu.subtract)
                        nc.vector.tensor_tensor(tzs, psI, wpR[rs_, cs], alu.mult)
                        nc.vector.tensor_tensor(prjI[rs_, cs], prjI[rs_, cs], tzs, alu.add)
                        psvR = psz_pool.tile([32, 512], f32, tag="Z")
                        nc.tensor.matmul(psvR, att, ryvR, start=True, stop=True)
                        nc.vector.scalar_tensor_tensor(
                            vskR[rs_, cs], psvR, wp_, vskR[rs_, cs], alu.mult, alu.add)
                        psvI = psz_pool.tile([32, 512], f32, tag="Z")
                        nc.tensor.matmul(psvI, att, ryvI, start=True, stop=True)
                        nc.vector.scalar_tensor_tensor(
                            vskI[rs_, cs], psvI, wp_, vskI[rs_, cs], alu.mult, alu.add)

            # ---- ReduceScatter ----
            for tl, col in ((prjR, 0), (prjI, 2048), (vskR, 4096), (vskI, 6144)):
                nc.sync.dma_start(rsin[:, col:col+2048], tl)
            zb = spool.tile([32, 32], f32, tag="zb")
            nc.vector.memset(zb, 0.0)
            nc.sync.dma_start(rsin[0:32, 8192:8224], beta0)
            nc.sync.dma_start(rsin[0:32, 8224:8256], zb)
            nc.sync.dma_start(rsin[32:64, 8192:8224], zb)
            nc.sync.dma_start(rsin[32:64, 8224:8256], beta1)
            nc.gpsimd.collective_compute(
                "ReduceScatter", mybir.AluOpType.add, GRP,
                ins=[rsin[:, :]], outs=[rsout[:, :]])

            # ---- W2 (local 8 tokens; global-token beta vs xfull) ----
            brows = spool.tile([8, 64], f32, tag="brows")
            nc.sync.dma_start(brows, rsout[:, 8192:8256])
            ps_bt = psa.tile([64, 8], f32, tag="A")
            nc.tensor.transpose(ps_bt, brows, ident[0:8, 0:8])
            bta = spool.tile([64, 8], f32, tag="bta")
            bta2 = spool.tile([64, 8], f32, tag="bta2")
            nc.scalar.mul(bta2, ps_bt, 1.0)
            nc.vector.tensor_scalar_mul(bta, bta2, stats[:, 2:3])
            ps_cc = psb.tile([8, 1], f32, tag="B")
            nc.tensor.matmul(ps_cc, bta2, stats[:, 3:4], start=True, stop=True)
            c8 = spool.tile([8, 1], f32, tag="c8")
            nc.vector.tensor_tensor(c8, ps_cc, cbc[0:8, 28:29], alu.add)
            for ch in range(8):
                xbc = opool.tile([64, 2048], f32, tag="xc")
                nc.sync.dma_start(xbc, xfull[:, 2048*ch:2048*ch+2048])
                for jj in range(4):
                    j = 4*ch + jj
                    ps_w = psb.tile([8, 512], f32, tag="B")
                    nc.tensor.matmul(ps_w, bta, xbc[:, 512*jj:512*jj+512],
                                     start=True, stop=True)
                    w2c = spool.tile([8, 512], f32, tag="w2c")
                    nc.vector.tensor_scalar_add(w2c, ps_w, c8[:, 0:1])
                    nc.sync.dma_start(w2s[:, 512*j:512*j+512], w2c)

            # ---- final per-token ----
            qr = qinv[:, 0:128]
            qi = qinv[:, 128:256]
            qin = qinv[:, 256:384]
            pr = pinv[:, 0:128]
            pin = pinv[:, 128:256]

            def inorm(dst, src, gcol, bcol, tagp):
                rsum = spool.tile([128, 2], f32, tag=f"rsum{tagp}")
                nc.vector.reduce_sum(rsum[:, 0:1], src, axis=X)
                sq = wpool.tile([128, 128], f32, tag="sqn")
                nc.scalar.activation(sq, src, AF.Square)
                nc.vector.reduce_sum(rsum[:, 1:2], sq, axis=X)
                ps_st = psc.tile([1, 2], f32, tag="C")
                nc.tensor.matmul(ps_st, ones, rsum, start=True, stop=True)
                st2 = spool.tile([1, 2], f32, tag=f"st2{tagp}")
                nc.vector.tensor_scalar_mul(st2, ps_st, 1.0/16384.0)
                vv = spool.tile([1, 2], f32, tag=f"vv{tagp}")
                nc.vector.tensor_tensor(vv[0:1, 0:1], st2[0:1, 0:1], st2[0:1, 0:1], alu.mult)
                nc.vector.tensor_tensor(vv[0:1, 1:2], st2[0:1, 1:2], vv[0:1, 0:1], alu.subtract)
                sa = spool.tile([1, 2], f32, tag=f"sa{tagp}")
                nc.scalar.activation(sa[0:1, 0:1], vv[0:1, 1:2], AF.Sqrt, bias=epst[0:1, 0:1])
                nc.vector.reciprocal(sa[0:1, 0:1], sa[0:1, 0:1])
                nc.vector.tensor_tensor(sa[0:1, 0:1], sa[0:1, 0:1], crow[0:1, gcol:gcol+1], alu.mult)
                nc.vector.tensor_tensor(vv[0:1, 0:1], st2[0:1, 0:1], sa[0:1, 0:1], alu.mult)
                nc.vector.tensor_tensor(sa[0:1, 1:2], crow[0:1, bcol:bcol+1], vv[0:1, 0:1], alu.subtract)
                sab = spool.tile([128, 2], f32, tag=f"sab{tagp}")
                nc.gpsimd.partition_broadcast(sab, sa[0:1, :])
                nc.vector.tensor_scalar(dst, src, sab[:, 0:1], sab[:, 1:2], alu.mult, alu.add)

            def inv32(mR_, mI_, psy, start, stop):
                psz = psa.tile([64, 256], f32, tag="A")
                nc.tensor.matmul(psz[:, 0:128], mR_, qr, start=True, stop=False)
                nc.tensor.matmul(psz[:, 0:128], mI_, qin, start=False, stop=True)
                nc.tensor.matmul(psz[:, 128:256], mR_, qi, start=True, stop=False)
                nc.tensor.matmul(psz[:, 128:256], mI_, qr, start=False, stop=True)
                zz = wpool.tile([64, 256], f32, tag="i_zz")
                nc.scalar.mul(zz, psz, 1.0)
                nc.tensor.matmul(psy, pr, zz[:, 0:128], start=start, stop=False)
                nc.tensor.matmul(psy, pin, zz[:, 128:256], start=False, stop=stop)

            def fwd_mix(src_sb, layer):
                ps1 = psa.tile([128, 128], f32, tag="A")
                nc.tensor.matmul(ps1, src_sb, frt2, start=True, stop=True)
                ct = wpool.tile([128, 128], f32, tag="f_ct")
                nc.scalar.mul(ct, ps1, 1.0)
                psm = psb.tile([64, 128], f32, tag="B")
                nc.tensor.matmul(psm, g2m, ct, start=True, stop=True)
                sbmA = wpool.tile([32, 128], f32, tag="f_sbmA")
                sbmB = wpool.tile([32, 128], f32, tag="f_sbmB")
                nc.scalar.mul(sbmA, psm[0:32, :], 1.0)
                nc.scalar.mul(sbmB, psm[32:64, :], 1.0)
                xm = wpool.tile([32, 128], f32, tag="f_xm")
                nc.vector.tensor_tensor(xm[:, 0:64], sbmA[:, 0:64],
                                        sbmB[:, 64:128], alu.subtract)
                nc.vector.tensor_tensor(xm[:, 64:128], sbmA[:, 64:128],
                                        sbmB[:, 0:64], alu.add)
                vm = wpool.tile([32, 128], f32, tag="f_vm")
                tm = wpool.tile([32, 64], f32, tag="f_tm")
                cR = cmix[:, 128*layer:128*layer+64]
                cI = cmix[:, 128*layer+64:128*layer+128]
                nc.vector.tensor_tensor(vm[:, 0:64], xm[:, 0:64], cR, alu.mult)
                nc.vector.tensor_tensor(tm, xm[:, 64:128], cI, alu.mult)
                nc.vector.tensor_tensor(vm[:, 0:64], vm[:, 0:64], tm, alu.subtract)
                nc.vector.tensor_tensor(vm[:, 64:128], xm[:, 0:64], cI, alu.mult)
                nc.vector.tensor_tensor(tm, xm[:, 64:128], cR, alu.mult)
                nc.vector.tensor_tensor(vm[:, 64:128], vm[:, 64:128], tm, alu.add)
                return vm

            for tl_ in range(8):
                rec = rsout[tl_]
                mR = wpool.tile([32, 64], f32, tag="mR")
                mI = wpool.tile([32, 64], f32, tag="mI")
                vR = wpool.tile([32, 64], f32, tag="vR")
                vI = wpool.tile([32, 64], f32, tag="vI")
                for tl2, off in ((mR, 0), (mI, 2048), (vR, 4096), (vI, 6144)):
                    nc.sync.dma_start(
                        tl2, rec[off:off+2048].rearrange("(n m) -> n m", n=32))
                psy = psb.tile([128, 128], f32, tag="B")
                inv32(mR, mI, psy, start=True, stop=False)
                inv32(vR, vI, psy, start=False, stop=True)
                w2t = wpool.tile([128, 128], f32, tag="w2t")
                nc.sync.dma_start(w2t, w2s[tl_].rearrange("(h w) -> h w", h=128))
                xt = wpool.tile([128, 128], f32, tag="xt")
                nc.sync.dma_start(xt, xs[tl_].rearrange("(h w) -> h w", h=128))
                att0 = wpool.tile([128, 128], f32, tag="att0")
                nc.vector.tensor_tensor(att0, psy, w2t, alu.add)
                nc.vector.tensor_tensor(att0, att0, xt, alu.add)
                attn_img = wpool.tile([128, 128], f32, tag="attn_img")
                inorm(attn_img, att0, 2, 3, "a")
                an = wpool.tile([128, 128], f32, tag="an")
                inorm(an, attn_img, 4, 5, "b")
                vm1 = fwd_mix(an, 0)
                psf = psb.tile([128, 128], f32, tag="B")
                inv32(vm1[:, 0:64], vm1[:, 64:128], psf, start=True, stop=True)
                fn1 = wpool.tile([128, 128], f32, tag="fn1")
                inorm(fn1, psf, 6, 7, "c")
                m1 = wpool.tile([128, 128], f32, tag="m1")
                nc.vector.scalar_tensor_tensor(m1, an, cbc[:, 29:30], fn1,
                                               alu.mult, alu.add)
                nc.vector.tensor_scalar_add(m1, m1, cbc[:, 30:31])
                m1g = wpool.tile([128, 128], f32, tag="m1g")
                nc.scalar.activation(m1g, m1, AF.Gelu)
                vm2 = fwd_mix(m1g, 1)
                psf2 = psb.tile([128, 128], f32, tag="B")
                inv32(vm2[:, 0:64], vm2[:, 64:128], psf2, start=True, stop=True)
                fn2 = wpool.tile([128, 128], f32, tag="fn2")
                inorm(fn2, psf2, 8, 9, "d")
                m2 = wpool.tile([128, 128], f32, tag="m2")
                nc.vector.scalar_tensor_tensor(m2, m1g, cbc[:, 31:32], fn2,
                                               alu.mult, alu.add)
                nc.vector.tensor_scalar_add(m2, m2, cbc[:, 32:33])
                oimg = wpool.tile([128, 128], f32, tag="oimg")
                inorm(oimg, m2, 10, 11, "e")
                nc.vector.tensor_tensor(oimg, oimg, attn_img, alu.add)
                nc.sync.dma_start(out[tl_].rearrange("(h w) -> h w", h=128), oimg)

    nc.compile()
    return nc


# ---------------------------------------------------------------------------
# kernel()
# ---------------------------------------------------------------------------

def kernel(x, wK, wKs, bKs, wQ, wQs, bQs, wV, wVs, bVs, wP, wPs, bPs,
           wM0, wM0s, bM0s, wM1, wM1s, bM1s, norm_g, norm_b):
    global _NC, LAST_EXEC_NS
    import concourse.bass_utils as bass_utils

    loc = dict(x=x, wK=wK, wKs=wKs, bKs=bKs, wQ=wQ, wQs=wQs, bQs=bQs,
               wV=wV, wVs=wVs, bVs=bVs, wP=wP, wPs=wPs, bPs=bPs,
               wM0=wM0, wM0s=wM0s, bM0s=bM0s, wM1=wM1, wM1s=wM1s, bM1s=bM1s,
               norm_g=norm_g, norm_b=norm_b)
    args = {k: np.asarray(v, np.float32) for k, v in loc.items()}
    C = _make_consts(args)
    x64 = args["x"].reshape(64, 16384)

    if _NC is None:
        _NC = _build_nc()

    wVs_l, wPs_l, bVs_l = C["_W"]
    in_maps = []
    for c in range(8):
        hs = slice(4*c, 4*c+4)
        crow = C["_crow_base"].copy()
        crow[0, 12:16] = wVs_l[hs]
        crow[0, 16:20] = wPs_l[hs]
        crow[0, 20:24] = wPs_l[hs]*wVs_l[hs]
        crow[0, 24:28] = bVs_l[hs]
        dcwm = np.zeros((3, 16), np.float32)
        for i, o in enumerate(range(4*c, 4*c+4)):
            dcwm[:, 4*i:4*i+4] = C["DCWM"][o]
        percore = {
            "FrT2": C["FrT2"], "G2": C["G2"], "G2m": C["G2m"],
            "IDENT": C["IDENT"], "QINV": C["QINV"], "PINV": C["PINV"],
            "PPERM": C["PPERM"], "CMIX": C["CMIX"], "ONES": C["ONES"],
            "DK": np.concatenate(
                [np.concatenate([C["DK"][o, 0], C["DK"][o, 1]], 1)
                 for o in range(4*c, 4*c+4)], 1),
            "EP": np.concatenate(
                [np.concatenate([C["EP"][o, 0], C["EP"][o, 1]], 1)
                 for o in range(4*c, 4*c+4)], 1),
            "DCWM": dcwm, "CROW": crow,
            "CVF": C["CVF"][hs].reshape(8, 2048),
            "WPF": C["WPF"][hs].reshape(8, 2048),
        }
        ct = np.concatenate(
            [np.asarray(percore[n], np.float32).ravel() for n, _ in CT_TABLES])
        in_maps.append({
            "xs": np.ascontiguousarray(x64[8*c:8*c+8]),
            "CT": np.ascontiguousarray(ct.reshape(1, CT_TOT)),
        })
    t0 = time.time()
    res = bass_utils.run_bass_kernel_spmd(_NC, in_maps, core_ids=list(range(8)))
    t1 = time.time()
    LAST_EXEC_NS = (res.exec_time_ns if res.exec_time_ns
                    else int((t1 - t0) * 1e9))
    outp = np.concatenate([np.asarray(r["out"]) for r in res.results], axis=0)
    return np.ascontiguousarray(outp.reshape(2, 32, 128, 128).astype(np.float32))


# revision 5
# speedup vs baseline: 102.0324x; 102.0324x over previous
"""Trainium2 kernel for nn_CODABlocks2D: full CODA block on-device in Fourier space.

Sharding: heads 4c..4c+3 on core c for K/Q/V/attention/proj (x replicated via
on-device AllGather); final per-token stages token-sharded 8/core via an
on-device ReduceScatter of projection partials.  Attention scores are computed
directly in truncated-mode space (Parseval); K/Q/V images are never
materialized.  I/O per core: x shard (0.5MB), const tables (~0.6MB), output
shard (0.5MB).

Mode layout: XfS/XnS [128 par, 2112]: partitions 0..63 = Re over m-rows
(SEL64), 64..127 = Im; free = (t major 64, n minor 33).
Token-major: XnP_tok [64 t, (n 32 major, m 64 minor)].
"""
import sys
import time

import numpy as np

sys.path.insert(0, "/opt/trn_rl_repo")

EPS = 1e-5
NH = 32
SEL64 = np.r_[0:32, 96:128]
NCOL = 33
PERM = np.r_[0, 64 - np.arange(1, 64)]

LAST_EXEC_NS = None
_NC = None


# ---------------------------------------------------------------------------
# Host constant tables
# ---------------------------------------------------------------------------

def _dft_mats():
    h = np.arange(128)
    FrT = np.exp(-2j*np.pi*np.outer(h, SEL64)/128) / (128.0*128.0)   # [128h,64m]
    G = np.exp(-2j*np.pi*np.outer(h, np.arange(NCOL))/128)           # [128w,33n]
    P = np.exp(2j*np.pi*np.outer(h, SEL64)/128)                      # [128h,64m]
    gam = np.ones(NCOL); gam[1:32] = 2.0
    Q = gam[:, None]*np.exp(2j*np.pi*np.outer(np.arange(NCOL), h)/128)
    Q[0, :] = Q[0, :].real
    return FrT, G, P, Q


def _conv_pad(w):
    wc = w[..., 0] + 1j*w[..., 1]
    C = np.zeros((NH, 64, NCOL), np.complex128)
    C[:, 0:16, 0:16] = wc[0, 0]
    C[:, 48:64, 0:16] = wc[1, 0]
    return C


def _make_consts(args):
    FrT, G, P, Q = _dft_mats()
    f32 = np.float32
    c = {}
    c["FrT2"] = np.ascontiguousarray(
        np.concatenate([FrT.real, FrT.imag], 1).astype(f32))
    c["G2"] = np.ascontiguousarray(
        np.concatenate([G.real, G.imag], 1).astype(f32))
    c["G2m"] = np.ascontiguousarray(
        np.concatenate([G.real[:, 0:32], G.imag[:, 0:32]], 1).astype(f32))
    c["IDENT"] = np.eye(128, dtype=f32)
    c["QINV"] = np.ascontiguousarray(np.concatenate(
        [Q[0:32].real, Q[0:32].imag, -Q[0:32].imag], 1).astype(f32))   # [32,384]
    c["PINV"] = np.ascontiguousarray(np.concatenate(
        [P.real.T, -P.imag.T], 1).astype(f32))                          # [64,256]
    Pp = np.zeros((128, 128), f32)
    for m in range(64):
        Pp[PERM[m], m] = 1.0
        Pp[64 + PERM[m], 64 + m] = 1.0
    c["PPERM"] = Pp

    CK = _conv_pad(args["wK"]) + args["wKs"][:, 0][:, None, None]
    CQ = _conv_pad(args["wQ"]) + args["wQs"][:, 0][:, None, None]
    CV = np.zeros((NH, 64, NCOL), np.complex128)
    wcV = args["wV"][..., 0] + 1j*args["wV"][..., 1]
    CV[:, 0:16, 0:16] = wcV[0, 0]
    CV[:, 48:64, 0:16] = wcV[1, 0]
    wcP = args["wP"][..., 0] + 1j*args["wP"][..., 1]
    WP = np.zeros((NH, 64, 32), np.complex128)
    WP[:, 0:32] = wcP[0, :, 0]
    WP[:, 32:64] = wcP[1, :, 0]

    alpha = np.full(NCOL, 128.0); alpha[0] = 32.0; alpha[32] = 32.0
    b0v = 32.0
    D = alpha[None, None, :]*CQ*np.conj(CK)
    D[:, 0, 0] = 0.0
    c["DK"] = np.stack([D.real, D.imag], 1).astype(f32)            # [NH,2,64,33]
    E0 = b0v*CQ[:, :, 0]*CK[:, PERM, 0]; E0[:, 0] = 0.0
    E32 = b0v*CQ[:, :, 32]*CK[:, PERM, 32]
    E = np.stack([E0, E32], -1)                                    # [NH,64,2]
    c["EP"] = np.stack([E.real, E.imag], 1).astype(f32)            # [NH,2,64,2]

    # DC ext: [XnRdc; XnIdc; 1] @ DCWM -> [qdR, qdI, kdR', kdI'] per head
    a0 = alpha[0]
    dcwm = np.zeros((NH, 3, 4), f32)
    dcwm[:, 0, 0] = CQ[:, 0, 0].real
    dcwm[:, 1, 0] = -CQ[:, 0, 0].imag
    dcwm[:, 2, 0] = args["bQs"]
    dcwm[:, 0, 1] = CQ[:, 0, 0].imag
    dcwm[:, 1, 1] = CQ[:, 0, 0].real
    dcwm[:, 0, 2] = (a0 + b0v)*CK[:, 0, 0].real
    dcwm[:, 1, 2] = -(a0 + b0v)*CK[:, 0, 0].imag
    dcwm[:, 2, 2] = (a0 + b0v)*args["bKs"]
    dcwm[:, 0, 3] = (a0 - b0v)*CK[:, 0, 0].imag
    dcwm[:, 1, 3] = (a0 - b0v)*CK[:, 0, 0].real
    c["DCWM"] = dcwm

    CVs = CV.copy()
    c0 = CVs[:, :, 0].copy()
    n0 = 0.5*(c0 + np.conj(c0[:, PERM]))
    n0[:, 32] = 0.5*c0[:, 32]
    n0[:, 0] = c0[:, 0].real
    CVs[:, :, 0] = n0
    CVf = np.transpose(CVs[:, :, 0:32], (0, 2, 1))                 # [NH,32n,64m]
    c["CVF"] = np.ascontiguousarray(
        np.stack([CVf.real, CVf.imag], 1).reshape(NH, 2, 2048).astype(f32))
    WPf = np.transpose(WP, (0, 2, 1))
    c["WPF"] = np.ascontiguousarray(
        np.stack([WPf.real, WPf.imag], 1).reshape(NH, 2, 2048).astype(f32))

    def mixw(w):
        wc = w[..., 0] + 1j*w[..., 1]
        M = np.zeros((64, 32), np.complex128)
        M[0:32] = wc[0, 0, 0]
        M[32:64] = wc[1, 0, 0]
        return M.T
    CM0, CM1 = mixw(args["wM0"]), mixw(args["wM1"])
    c["CMIX"] = np.ascontiguousarray(np.concatenate(
        [CM0.real, CM0.imag, CM1.real, CM1.imag], 1).astype(f32))  # [32,256]
    c["ONES"] = np.ones((128, 1), f32)

    g, bb = args["norm_g"], args["norm_b"]
    const1 = float(args["bPs"][0] + np.dot(args["wPs"][0], args["bVs"]))
    crow = np.zeros((1, 64), f32)
    for k, v in {0: g[0], 1: bb[0], 2: g[1], 3: bb[1], 4: g[2], 5: bb[2],
                 6: g[3], 7: bb[3], 8: g[4], 9: bb[4], 10: g[5], 11: bb[5],
                 28: const1, 29: args["wM0s"][0, 0], 30: args["bM0s"][0],
                 31: args["wM1s"][0, 0], 32: args["bM1s"][0]}.items():
        crow[0, k] = v
    c["_crow_base"] = crow
    c["_W"] = (args["wVs"][:, 0].astype(f32), args["wPs"][0].astype(f32),
               args["bVs"].astype(f32))
    return c


# (name, shape) layout of the consolidated const tensor; per-core entries
# (DK/EP/DCWM/CROW/CVF/WPF) are appended by kernel() per core.
CT_TABLES = [("FrT2", (128, 128)), ("G2", (128, 66)), ("G2m", (128, 64)),
             ("IDENT", (128, 128)), ("QINV", (32, 384)), ("PINV", (64, 256)),
             ("PPERM", (128, 128)), ("CMIX", (32, 256)), ("ONES", (128, 1)),
             ("DK", (64, 264)), ("EP", (64, 16)), ("DCWM", (3, 16)),
             ("CROW", (1, 64)), ("CVF", (8, 2048)), ("WPF", (8, 2048))]
CT_OFF = {}
_o = 0
for _n, _sh in CT_TABLES:
    CT_OFF[_n] = _o
    _o += _sh[0]*_sh[1]
CT_TOT = _o


# ---------------------------------------------------------------------------
# Bass kernel
# ---------------------------------------------------------------------------

def _build_nc():
    import concourse.bacc as bacc
    import concourse.mybir as mybir
    from concourse.tile import TileContext
    from concourse.bass import AP
    from concourse.alu_op_type import AluOpType as alu

    f32 = mybir.dt.float32
    bf16 = mybir.dt.bfloat16
    X = mybir.AxisListType.X
    AF = mybir.ActivationFunctionType
    GRP = [[0, 1, 2, 3, 4, 5, 6, 7]]
    REC = 8256

    nc = bacc.Bacc(None, target_bir_lowering=False)
    dt = nc.dram_tensor
    xs = dt("xs", [8, 16384], bf16, kind="ExternalInput")
    CT = dt("CT", [1, CT_TOT], bf16, kind="ExternalInput")

    xsi = dt("xsi", [8, 16384], bf16, kind="Internal")
    xfull = dt("xfull", [64, 16384], bf16, kind="Internal", addr_space="Shared")
    rsin = dt("rsin", [64, REC], f32, kind="Internal")
    rsout = dt("rsout", [8, REC], f32, kind="Internal")
    w2s = dt("w2s", [8, 16384], f32, kind="Internal")
    out = dt("out", [8, 16384], f32, kind="ExternalOutput")

    with TileContext(nc) as tc:
        with tc.tile_pool(name="const", bufs=1) as cpool, \
             tc.tile_pool(name="big", bufs=1) as bpool, \
             tc.tile_pool(name="wrk", bufs=2) as wpool, \
             tc.tile_pool(name="one", bufs=1) as opool, \
             tc.tile_pool(name="strip", bufs=2) as spool, \
             tc.tile_pool(name="ps_a", bufs=2, space="PSUM") as psa, \
             tc.tile_pool(name="ps_b", bufs=2, space="PSUM") as psb, \
             tc.tile_pool(name="ps_c", bufs=2, space="PSUM") as psc, \
             tc.tile_pool(name="ps_z", bufs=2, space="PSUM") as psz_pool:

            ctb = CT[0:1, :]

            def cload(name):
                rows, width = dict(CT_TABLES)[name]
                stg = wpool.tile([128, 384], bf16, tag="ctstg")
                nc.sync.dma_start(
                    stg[0:rows, 0:width],
                    AP(ctb.tensor, CT_OFF[name], [[width, rows], [1, width]]))
                t = cpool.tile([rows, width], f32, tag=name)
                nc.vector.tensor_copy(t, stg[0:rows, 0:width])
                return t
            frt2 = cload("FrT2")
            g2 = cload("G2")
            g2m = cload("G2m")
            ident = cload("IDENT")
            qinv = cload("QINV")
            pinv = cload("PINV")
            pperm = cload("PPERM")
            dk = cload("DK")
            ept = cload("EP")
            dcwm = cload("DCWM")
            cmix = cload("CMIX")
            ones = cload("ONES")
            crow = cload("CROW")
            cbc = cpool.tile([128, 64], f32, tag="cbc")
            nc.gpsimd.partition_broadcast(cbc, crow[0:1, :])
            epst = cpool.tile([128, 1], f32, tag="epst")
            nc.vector.memset(epst, EPS)

            # ---- AllGather x (stage via SBUF chunks; collectives can't read IO) ----
            for ch in range(16):
                xg = opool.tile([8, 1024], bf16, tag="xg")
                nc.sync.dma_start(xg, xs[:, 1024*ch:1024*ch+1024])
                nc.sync.dma_start(xsi[:, 1024*ch:1024*ch+1024], xg)
            nc.gpsimd.collective_compute(
                "AllGather", mybir.AluOpType.bypass, GRP,
                ins=[xsi[:, :]], outs=[xfull[:, :]])

            # ---- stats over all 64 tokens (chunked) ----
            s12 = cpool.tile([64, 2], f32, tag="s12")
            nc.vector.memset(s12, 0.0)
            for ch in range(8):
                xcb = opool.tile([64, 2048], bf16, tag="xcb")
                nc.sync.dma_start(xcb, xfull[:, 2048*ch:2048*ch+2048])
                xc = opool.tile([64, 2048], f32, tag="xc")
                nc.vector.tensor_copy(xc, xcb)
                pt = spool.tile([64, 2], f32, tag="pt")
                nc.vector.reduce_sum(pt[:, 0:1], xc, axis=X)
                nc.vector.tensor_tensor(xc, xc, xc, alu.mult)
                nc.vector.reduce_sum(pt[:, 1:2], xc, axis=X)
                nc.vector.tensor_tensor(s12, s12, pt, alu.add)
            stats = cpool.tile([64, 4], f32, tag="stats")  # mu, var, a, c
            nc.vector.tensor_scalar_mul(stats[:, 0:2], s12, 1.0/16384.0)
            musq = spool.tile([64, 1], f32, tag="musq")
            nc.vector.tensor_tensor(musq, stats[:, 0:1], stats[:, 0:1], alu.mult)
            nc.vector.tensor_tensor(stats[:, 1:2], stats[:, 1:2], musq, alu.subtract)
            rsv = spool.tile([64, 1], f32, tag="rsv")
            nc.scalar.activation(rsv, stats[:, 1:2], AF.Sqrt, bias=epst[0:64, 0:1])
            nc.vector.reciprocal(rsv, rsv)
            nc.vector.tensor_tensor(stats[:, 2:3], rsv, cbc[0:64, 0:1], alu.mult)
            nc.vector.tensor_tensor(musq, stats[:, 0:1], stats[:, 2:3], alu.mult)
            nc.vector.tensor_tensor(stats[:, 3:4], cbc[0:64, 1:2], musq, alu.subtract)
            ps_a1 = psa.tile([1, 64], f32, tag="A")
            nc.tensor.transpose(ps_a1, stats[:, 2:3], ident[0:64, 0:64])
            arow1 = cpool.tile([1, 64], f32, tag="arow1")
            nc.scalar.mul(arow1, ps_a1, 1.0)
            ps_c1 = psa.tile([1, 64], f32, tag="A")
            nc.tensor.transpose(ps_c1, stats[:, 3:4], ident[0:64, 0:64])
            crow2 = cpool.tile([1, 64], f32, tag="crow2")
            nc.scalar.mul(crow2, ps_c1, 1.0)
            # A_bc [128, 2112], layout (t, n): value a_t
            arep = cpool.tile([1, 2112], f32, tag="arep")
            sa_ = arow1[0:1, :]
            nc.vector.tensor_copy(
                arep.rearrange("p (t n) -> p t n", n=33),
                AP(sa_.tensor, sa_.offset, [sa_.ap[0], [1, 64], [0, 33]]))
            abc = cpool.tile([128, 2112], f32, tag="abc")
            nc.gpsimd.partition_broadcast(abc, arep[0:1, :])

            # ---- forward DFT per token -> xfs [128, 2112] (t,n layout) ----
            xfs = bpool.tile([128, 2112], f32, tag="xfs")
            for t in range(64):
                xtb = wpool.tile([128, 128], bf16, tag="xtb")
                nc.sync.dma_start(
                    xtb, xfull[t].rearrange("(h w) -> h w", h=128))
                xtile = wpool.tile([128, 128], f32, tag="xtile")
                nc.vector.tensor_copy(xtile, xtb)
                ps1 = psa.tile([128, 128], f32, tag="A")
                nc.tensor.matmul(ps1, xtile, frt2, start=True, stop=True)
                ct2 = wpool.tile([128, 128], f32, tag="ct2")
                nc.scalar.mul(ct2, ps1, 1.0)
                ps2 = psb.tile([128, 66], f32, tag="B")
                nc.tensor.matmul(ps2, ct2, g2, start=True, stop=True)
                sbA = wpool.tile([64, 66], f32, tag="sbA")
                sbB = wpool.tile([64, 66], f32, tag="sbB")
                nc.scalar.mul(sbA, ps2[0:64, :], 1.0)
                nc.scalar.mul(sbB, ps2[64:128, :], 1.0)
                nc.vector.tensor_tensor(xfs[0:64, 33*t:33*t+33],
                                        sbA[:, 0:33], sbB[:, 33:66],
                                        alu.subtract)
                nc.vector.tensor_tensor(xfs[64:128, 33*t:33*t+33],
                                        sbA[:, 33:66], sbB[:, 0:33],
                                        alu.add)

            # ---- normalize ----
            xns = bpool.tile([128, 2112], f32, tag="xns")
            nc.vector.tensor_tensor(xns, xfs, abc, alu.mult)
            bdc = xns[0:1, :]
            dcap = AP(bdc.tensor, bdc.offset, [bdc.ap[0], [33, 64]])
            nc.vector.tensor_tensor(dcap, dcap, crow2, alu.add)
            # base-0 copy of the Im half (same-base-partition rule for DVE)
            xnsI2 = abc[0:64, :]
            nc.vector.tensor_copy(xnsI2, xns[64:128, :])

            # ---- permuted strips (n=0, n=32) ----
            xnstr = cpool.tile([128, 128], f32, tag="xnstr")
            bx = xns[:, :]
            for j, n in ((0, 0), (1, 32)):
                nc.vector.tensor_copy(
                    xnstr[:, 64*j:64*j+64],
                    AP(bx.tensor, bx.offset + n, [bx.ap[0], [33, 64]]))
            ps_p = psa.tile([128, 128], f32, tag="A")
            nc.tensor.matmul(ps_p, pperm, xnstr, start=True, stop=True)
            xnps = cpool.tile([128, 128], f32, tag="xnps")
            nc.scalar.mul(xnps, ps_p, 1.0)
            xnpsI = cpool.tile([64, 128], f32, tag="xnpsI")
            nc.vector.tensor_copy(xnpsI, xnps[64:128, :])

            # ---- XnP_tok ----
            xnpR = bpool.tile([64, 2048], f32, tag="xnpR")
            xnpI = bpool.tile([64, 2048], f32, tag="xnpI")
            for n in range(32):
                ps_t = psc.tile([64, 128], f32, tag="C")
                nc.tensor.transpose(
                    ps_t,
                    AP(bx.tensor, bx.offset + n, [bx.ap[0], [33, 64]]),
                    ident)
                nc.scalar.mul(xnpR[:, 64*n:64*n+64], ps_t[:, 0:64], 1.0)
                nc.scalar.mul(xnpI[:, 64*n:64*n+64], ps_t[:, 64:128], 1.0)

            # ---- DC strips gather (partitions 0 and 64 -> [3, 64]) ----
            dcstr = spool.tile([3, 64], f32, tag="dcstr")
            nc.vector.memset(dcstr, 1.0)
            nc.sync.dma_start(
                dcstr[0:2, :],
                AP(bx.tensor, bx.offset, [[bx.ap[0][0]*64, 2], [33, 64]]))

            # ---- accumulators ----
            prjR = bpool.tile([64, 2048], f32, tag="prjR")
            prjI = bpool.tile([64, 2048], f32, tag="prjI")
            vskR = bpool.tile([64, 2048], f32, tag="vskR")
            vskI = bpool.tile([64, 2048], f32, tag="vskI")
            beta0 = cpool.tile([32, 32], f32, tag="beta0")
            beta1 = cpool.tile([32, 32], f32, tag="beta1")
            for tl in (prjR, prjI, vskR, vskI, beta0, beta1):
                nc.vector.memset(tl, 0.0)

            def tn(tile_ap, n, b):
                return AP(tile_ap.tensor, tile_ap.offset + 33*32*b + n,
                          [tile_ap.ap[0], [33, 32]])

            iw = {"wVs": 12, "wPs": 16, "wPwV": 20, "bVs": 24}
            attn_all = {}
            for o in range(4):
                # K-side main [128, 2112]; all products at base 0, then copy Im half
                ks = opool.tile([128, 2112], f32, tag="ks")
                v3 = lambda tl: tl.rearrange("p (t n) -> p t n", n=33)
                dkR = dk[:, 66*o:66*o+33]
                dkI = dk[:, 66*o+33:66*o+66]
                bc33 = lambda w: AP(w.tensor, w.offset, [w.ap[0], [0, 64], [1, 33]])
                xnR = xns[0:64, :]
                tmp0 = xfs[0:64, :]     # xfs is dead after normalization; scratch
                ksr = ks[0:64, :]
                # Im half first (built in ks[0:64], copied up)
                nc.vector.tensor_tensor(v3(ksr), v3(xnsI2), bc33(dkR), alu.mult)
                nc.vector.tensor_tensor(v3(tmp0), v3(xnR), bc33(dkI), alu.mult)
                nc.vector.tensor_tensor(ksr, ksr, tmp0, alu.subtract)
                nc.vector.tensor_copy(ks[64:128, :], ksr)
                # Re half
                nc.vector.tensor_tensor(v3(ksr), v3(xnR), bc33(dkR), alu.mult)
                nc.vector.tensor_tensor(v3(tmp0), v3(xnsI2), bc33(dkI), alu.mult)
                nc.vector.tensor_tensor(ksr, ksr, tmp0, alu.add)
                # K-perm [128, 128] layout (j, t)
                kp = wpool.tile([128, 128], f32, tag="kp")
                tpa = wpool.tile([64, 128], f32, tag="tpa")
                v2 = lambda tl: tl.rearrange("p (j t) -> p j t", t=64)
                epR = ept[:, 4*o:4*o+2]
                epI = ept[:, 4*o+2:4*o+4]
                bc2 = lambda w: AP(w.tensor, w.offset, [w.ap[0], [1, 2], [0, 64]])
                xpR = xnps[0:64, :]
                kpr = kp[0:64, :]
                # Im half = -(ER*XnpI + EI*XnpR), built low then copied up
                nc.vector.tensor_tensor(v2(kpr), v2(xnpsI), bc2(epR), alu.mult)
                nc.vector.tensor_tensor(v2(tpa), v2(xpR), bc2(epI), alu.mult)
                nc.vector.tensor_tensor(kpr, kpr, tpa, alu.add)
                nc.vector.tensor_scalar_mul(kpr, kpr, -1.0)
                nc.vector.tensor_copy(kp[64:128, :], kpr)
                # Re half = ER*XnpR - EI*XnpI
                nc.vector.tensor_tensor(v2(kpr), v2(xpR), bc2(epR), alu.mult)
                nc.vector.tensor_tensor(v2(tpa), v2(xnpsI), bc2(epI), alu.mult)
                nc.vector.tensor_tensor(kpr, kpr, tpa, alu.subtract)
                # DC ext [4, 64] = ([3,64]^T @ dcwm-slice)^T ... via matmul:
                ps_dq = psc.tile([2, 64], f32, tag="C")
                nc.tensor.matmul(ps_dq, dcwm[:, 4*o:4*o+2], dcstr, start=True, stop=True)
                qext = spool.tile([2, 64], f32, tag="qext")
                nc.vector.tensor_copy(qext, ps_dq)
                ps_dk = psc.tile([2, 64], f32, tag="C")
                nc.tensor.matmul(ps_dk, dcwm[:, 4*o+2:4*o+4], dcstr, start=True, stop=True)
                kext = spool.tile([2, 64], f32, tag="kext")
                nc.vector.tensor_copy(kext, ps_dk)
                for b in range(2):
                    ps_s = psb.tile([32, 32], f32, tag="B")
                    for n in range(33):
                        nc.tensor.matmul(ps_s, tn(bx, n, b), tn(ks[:, :], n, b),
                                         start=(n == 0), stop=False)
                    for j in range(2):
                        nc.tensor.matmul(ps_s, xnstr[:, 64*j+32*b:64*j+32*b+32],
                                         kp[:, 64*j+32*b:64*j+32*b+32],
                                         start=False, stop=False)
                    nc.tensor.matmul(ps_s, qext[:, 32*b:32*b+32],
                                     kext[:, 32*b:32*b+32], start=False, stop=True)
                    sc = spool.tile([32, 32], f32, tag="sc")
                    nc.scalar.mul(sc, ps_s, 1.0)
                    mx = spool.tile([32, 1], f32, tag="mx")
                    nc.vector.reduce_max(mx, sc, axis=X)
                    nmx = spool.tile([32, 1], f32, tag="nmx")
                    nc.vector.tensor_scalar_mul(nmx, mx, -1.0)
                    exv = spool.tile([32, 32], f32, tag="exv")
                    nc.scalar.activation(exv, sc, AF.Exp, bias=nmx[:, 0:1])
                    smv = spool.tile([32, 1], f32, tag="smv")
                    nc.vector.reduce_sum(smv, exv, axis=X)
                    rcv = spool.tile([32, 1], f32, tag="rcv")
                    nc.vector.reciprocal(rcv, smv)
                    at = spool.tile([32, 32], f32, tag=f"at{o}{b}")
                    nc.vector.tensor_scalar_mul(at, exv, rcv[:, 0:1])
                    ps_tt = psc.tile([32, 32], f32, tag="C")
                    nc.tensor.transpose(ps_tt, at, ident[0:32, 0:32])
                    att = spool.tile([32, 32], f32, tag=f"att{o}{b}")
                    nc.scalar.mul(att, ps_tt, 1.0)
                    attn_all[(o, b)] = (at, att)

            # ---- per-head Y build + mixes ----
            for o in range(4):
                cvR = opool.tile([64, 2048], f32, tag="cvR")
                cvI = opool.tile([64, 2048], f32, tag="cvI")
                wpR = opool.tile([64, 2048], f32, tag="wpR")
                wpI = opool.tile([64, 2048], f32, tag="wpI")
                for dst_, base_, row in ((cvR, CT_OFF["CVF"], 2*o),
                                         (cvI, CT_OFF["CVF"], 2*o+1),
                                         (wpR, CT_OFF["WPF"], 2*o),
                                         (wpI, CT_OFF["WPF"], 2*o+1)):
                    cvs = opool.tile([64, 2048], bf16, tag="xcb")
                    nc.sync.dma_start(
                        cvs, AP(ctb.tensor, base_ + 2048*row,
                                [[0, 64], [1, 2048]]))
                    nc.vector.tensor_copy(dst_, cvs)
                for b in range(2):
                    rs_ = slice(32*b, 32*b+32)
                    wv = cbc[rs_, iw["wVs"]+o:iw["wVs"]+o+1]
                    bv = cbc[rs_, iw["bVs"]+o:iw["bVs"]+o+1]
                    wp_ = cbc[rs_, iw["wPs"]+o:iw["wPs"]+o+1]
                    wpwv = cbc[0:32, iw["wPwV"]+o:iw["wPwV"]+o+1]
                    at, att = attn_all[(o, b)]
                    bt_ = beta1 if b else beta0
                    nc.vector.scalar_tensor_tensor(
                        bt_, at, wpwv, bt_, alu.mult, alu.add)
                    for j in range(4):
                        cs = slice(512*j, 512*j+512)
                        xR = xnpR[rs_, cs]
                        xI = xnpI[rs_, cs]
                        yvR = opool.tile([64, 512], f32, tag="yvR")
                        yvI = opool.tile([64, 512], f32, tag="yvI")
                        tya = opool.tile([64, 512], f32, tag="tya")
                        yR = opool.tile([64, 512], f32, tag="yR")
                        yI = opool.tile([64, 512], f32, tag="yI")
                        nc.vector.tensor_tensor(yvR[rs_, :], xR, cvR[rs_, cs], alu.mult)
                        nc.vector.tensor_tensor(tya[rs_, :], xI, cvI[rs_, cs], alu.mult)
                        nc.vector.tensor_tensor(yvR[rs_, :], yvR[rs_, :], tya[rs_, :], alu.subtract)
                        nc.vector.tensor_tensor(yvI[rs_, :], xR, cvI[rs_, cs], alu.mult)
                        nc.vector.tensor_tensor(tya[rs_, :], xI, cvR[rs_, cs], alu.mult)
                        nc.vector.tensor_tensor(yvI[rs_, :], yvI[rs_, :], tya[rs_, :], alu.add)
                        nc.vector.scalar_tensor_tensor(
                            yR[rs_, :], xR, wv, yvR[rs_, :], alu.mult, alu.add)
                        nc.vector.scalar_tensor_tensor(
                            yI[rs_, :], xI, wv, yvI[rs_, :], alu.mult, alu.add)
                        if j == 0:
                            nc.vector.tensor_scalar_add(
                                yR[rs_, 0:1], yR[rs_, 0:1], bv)
                        if b == 0:
                            ryR, ryI = yR[0:32, :], yI[0:32, :]
                            ryvR, ryvI = yvR[0:32, :], yvI[0:32, :]
                        else:
                            ryR = opool.tile([32, 512], f32, tag="ryR")
                            ryI = opool.tile([32, 512], f32, tag="ryI")
                            ryvR = opool.tile([32, 512], f32, tag="ryvR")
                            ryvI = opool.tile([32, 512], f32, tag="ryvI")
                            nc.vector.tensor_copy(ryR, yR[32:64, :])
                            nc.vector.tensor_copy(ryI, yI[32:64, :])
                            nc.vector.tensor_copy(ryvR, yvR[32:64, :])
                            nc.vector.tensor_copy(ryvI, yvI[32:64, :])
                        tz = opool.tile([64, 512], f32, tag="tz")
                        tzs = tz[rs_, :]
                        psR = psz_pool.tile([32, 512], f32, tag="Z")
                        nc.tensor.matmul(psR, att, ryR, start=True, stop=True)
                        nc.vector.tensor_tensor(tzs, psR, wpR[rs_, cs], alu.mult)
                        nc.vector.tensor_tensor(prjR[rs_, cs], prjR[rs_, cs], tzs, alu.add)
                        nc.vector.tensor_tensor(tzs, psR, wpI[rs_, cs], alu.mult)
                        nc.vector.tensor_tensor(prjI[rs_, cs], prjI[rs_, cs], tzs, alu.add)
                        psI = psz_pool.tile([32, 512], f32, tag="Z")
                        nc.tensor.matmul(psI, att, ryI, start=True, stop=True)
                        nc.vector.tensor_tensor(tzs, psI, wpI[rs_, cs], alu.mult)
                        nc.vector.tensor_tensor(prjR[rs_, cs], prjR[rs_, cs], tzs, alu.subtract)
                        nc.vector.tensor_tensor(tzs, psI, wpR[rs_, cs], alu.mult)
                        nc.vector.tensor_tensor(prjI[rs_, cs], prjI[rs_, cs], tzs, alu.add)
                        psvR = psz_pool.tile([32, 512], f32, tag="Z")
                        nc.tensor.matmul(psvR, att, ryvR, start=True, stop=True)
                        nc.vector.scalar_tensor_tensor(
                            vskR[rs_, cs], psvR, wp_, vskR[rs_, cs], alu.mult, alu.add)
                        psvI = psz_pool.tile([32, 512], f32, tag="Z")
                        nc.tensor.matmul(psvI, att, ryvI, start=True, stop=True)
                        nc.vector.scalar_tensor_tensor(
                            vskI[rs_, cs], psvI, wp_, vskI[rs_, cs], alu.mult, alu.add)

            # ---- ReduceScatter ----
            for tl, col in ((prjR, 0), (prjI, 2048), (vskR, 4096), (vskI, 6144)):
                nc.sync.dma_start(rsin[:, col:col+2048], tl)
            zb = spool.tile([32, 32], f32, tag="zb")
            nc.vector.memset(zb, 0.0)
            nc.sync.dma_start(rsin[0:32, 8192:8224], beta0)
            nc.sync.dma_start(rsin[0:32, 8224:8256], zb)
            nc.sync.dma_start(rsin[32:64, 8192:8224], zb)
            nc.sync.dma_start(rsin[32:64, 8224:8256], beta1)
            nc.gpsimd.collective_compute(
                "ReduceScatter", mybir.AluOpType.add, GRP,
                ins=[rsin[:, :]], outs=[rsout[:, :]])

            # ---- W2 (local 8 tokens; global-token beta vs xfull) ----
            brows = spool.tile([8, 64], f32, tag="brows")
            nc.sync.dma_start(brows, rsout[:, 8192:8256])
            ps_bt = psa.tile([64, 8], f32, tag="A")
            nc.tensor.transpose(ps_bt, brows, ident[0:8, 0:8])
            bta = spool.tile([64, 8], f32, tag="bta")
            bta2 = spool.tile([64, 8], f32, tag="bta2")
            nc.scalar.mul(bta2, ps_bt, 1.0)
            nc.vector.tensor_scalar_mul(bta, bta2, stats[:, 2:3])
            ps_cc = psb.tile([8, 1], f32, tag="B")
            nc.tensor.matmul(ps_cc, bta2, stats[:, 3:4], start=True, stop=True)
            c8 = spool.tile([8, 1], f32, tag="c8")
            nc.vector.tensor_tensor(c8, ps_cc, cbc[0:8, 28:29], alu.add)
            for ch in range(8):
                xbb = opool.tile([64, 2048], bf16, tag="xcb")
                nc.sync.dma_start(xbb, xfull[:, 2048*ch:2048*ch+2048])
                xbc = opool.tile([64, 2048], f32, tag="xc")
                nc.vector.tensor_copy(xbc, xbb)
                for jj in range(4):
                    j = 4*ch + jj
                    ps_w = psb.tile([8, 512], f32, tag="B")
                    nc.tensor.matmul(ps_w, bta, xbc[:, 512*jj:512*jj+512],
                                     start=True, stop=True)
                    w2c = spool.tile([8, 512], f32, tag="w2c")
                    nc.vector.tensor_scalar_add(w2c, ps_w, c8[:, 0:1])
                    nc.sync.dma_start(w2s[:, 512*j:512*j+512], w2c)

            # ---- final per-token ----
            qr = qinv[:, 0:128]
            qi = qinv[:, 128:256]
            qin = qinv[:, 256:384]
            pr = pinv[:, 0:128]
            pin = pinv[:, 128:256]

            def inorm(dst, src, gcol, bcol, tagp):
                rsum = spool.tile([128, 2], f32, tag=f"rsum{tagp}")
                nc.vector.reduce_sum(rsum[:, 0:1], src, axis=X)
                sq = wpool.tile([128, 128], f32, tag="sqn")
                nc.scalar.activation(sq, src, AF.Square)
                nc.vector.reduce_sum(rsum[:, 1:2], sq, axis=X)
                ps_st = psc.tile([1, 2], f32, tag="C")
                nc.tensor.matmul(ps_st, ones, rsum, start=True, stop=True)
                st2 = spool.tile([1, 2], f32, tag=f"st2{tagp}")
                nc.vector.tensor_scalar_mul(st2, ps_st, 1.0/16384.0)
                vv = spool.tile([1, 2], f32, tag=f"vv{tagp}")
                nc.vector.tensor_tensor(vv[0:1, 0:1], st2[0:1, 0:1], st2[0:1, 0:1], alu.mult)
                nc.vector.tensor_tensor(vv[0:1, 1:2], st2[0:1, 1:2], vv[0:1, 0:1], alu.subtract)
                sa = spool.tile([1, 2], f32, tag=f"sa{tagp}")
                nc.scalar.activation(sa[0:1, 0:1], vv[0:1, 1:2], AF.Sqrt, bias=epst[0:1, 0:1])
                nc.vector.reciprocal(sa[0:1, 0:1], sa[0:1, 0:1])
                nc.vector.tensor_tensor(sa[0:1, 0:1], sa[0:1, 0:1], crow[0:1, gcol:gcol+1], alu.mult)
                nc.vector.tensor_tensor(vv[0:1, 0:1], st2[0:1, 0:1], sa[0:1, 0:1], alu.mult)
                nc.vector.tensor_tensor(sa[0:1, 1:2], crow[0:1, bcol:bcol+1], vv[0:1, 0:1], alu.subtract)
                sab = spool.tile([128, 2], f32, tag=f"sab{tagp}")
                nc.gpsimd.partition_broadcast(sab, sa[0:1, :])
                nc.vector.tensor_scalar(dst, src, sab[:, 0:1], sab[:, 1:2], alu.mult, alu.add)

            def inv32(mR_, mI_, psy, start, stop):
                psz = psa.tile([64, 256], f32, tag="A")
                nc.tensor.matmul(psz[:, 0:128], mR_, qr, start=True, stop=False)
                nc.tensor.matmul(psz[:, 0:128], mI_, qin, start=False, stop=True)
                nc.tensor.matmul(psz[:, 128:256], mR_, qi, start=True, stop=False)
                nc.tensor.matmul(psz[:, 128:256], mI_, qr, start=False, stop=True)
                zz = wpool.tile([64, 256], f32, tag="i_zz")
                nc.scalar.mul(zz, psz, 1.0)
                nc.tensor.matmul(psy, pr, zz[:, 0:128], start=start, stop=False)
                nc.tensor.matmul(psy, pin, zz[:, 128:256], start=False, stop=stop)

            def fwd_mix(src_sb, layer):
                ps1 = psa.tile([128, 128], f32, tag="A")
                nc.tensor.matmul(ps1, src_sb, frt2, start=True, stop=True)
                ct = wpool.tile([128, 128], f32, tag="f_ct")
                nc.scalar.mul(ct, ps1, 1.0)
                psm = psb.tile([64, 128], f32, tag="B")
                nc.tensor.matmul(psm, g2m, ct, start=True, stop=True)
                sbmA = wpool.tile([32, 128], f32, tag="f_sbmA")
                sbmB = wpool.tile([32, 128], f32, tag="f_sbmB")
                nc.scalar.mul(sbmA, psm[0:32, :], 1.0)
                nc.scalar.mul(sbmB, psm[32:64, :], 1.0)
                xm = wpool.tile([32, 128], f32, tag="f_xm")
                nc.vector.tensor_tensor(xm[:, 0:64], sbmA[:, 0:64],
                                        sbmB[:, 64:128], alu.subtract)
                nc.vector.tensor_tensor(xm[:, 64:128], sbmA[:, 64:128],
                                        sbmB[:, 0:64], alu.add)
                vm = wpool.tile([32, 128], f32, tag="f_vm")
                tm = wpool.tile([32, 64], f32, tag="f_tm")
                cR = cmix[:, 128*layer:128*layer+64]
                cI = cmix[:, 128*layer+64:128*layer+128]
                nc.vector.tensor_tensor(vm[:, 0:64], xm[:, 0:64], cR, alu.mult)
                nc.vector.tensor_tensor(tm, xm[:, 64:128], cI, alu.mult)
                nc.vector.tensor_tensor(vm[:, 0:64], vm[:, 0:64], tm, alu.subtract)
                nc.vector.tensor_tensor(vm[:, 64:128], xm[:, 0:64], cI, alu.mult)
                nc.vector.tensor_tensor(tm, xm[:, 64:128], cR, alu.mult)
                nc.vector.tensor_tensor(vm[:, 64:128], vm[:, 64:128], tm, alu.add)
                return vm

            for tl_ in range(8):
                rec = rsout[tl_]
                mR = wpool.tile([32, 64], f32, tag="mR")
                mI = wpool.tile([32, 64], f32, tag="mI")
                vR = wpool.tile([32, 64], f32, tag="vR")
                vI = wpool.tile([32, 64], f32, tag="vI")
                for tl2, off in ((mR, 0), (mI, 2048), (vR, 4096), (vI, 6144)):
                    nc.sync.dma_start(
                        tl2, rec[off:off+2048].rearrange("(n m) -> n m", n=32))
                psy = psb.tile([128, 128], f32, tag="B")
                inv32(mR, mI, psy, start=True, stop=False)
                inv32(vR, vI, psy, start=False, stop=True)
                w2t = wpool.tile([128, 128], f32, tag="w2t")
                nc.sync.dma_start(w2t, w2s[tl_].rearrange("(h w) -> h w", h=128))
                xtb2 = wpool.tile([128, 128], bf16, tag="xtb")
                nc.sync.dma_start(xtb2, xs[tl_].rearrange("(h w) -> h w", h=128))
                xt = wpool.tile([128, 128], f32, tag="xt")
                nc.vector.tensor_copy(xt, xtb2)
                att0 = wpool.tile([128, 128], f32, tag="att0")
                nc.vector.tensor_tensor(att0, psy, w2t, alu.add)
                nc.vector.tensor_tensor(att0, att0, xt, alu.add)
                attn_img = wpool.tile([128, 128], f32, tag="attn_img")
                inorm(attn_img, att0, 2, 3, "a")
                an = wpool.tile([128, 128], f32, tag="an")
                inorm(an, attn_img, 4, 5, "b")
                vm1 = fwd_mix(an, 0)
                psf = psb.tile([128, 128], f32, tag="B")
                inv32(vm1[:, 0:64], vm1[:, 64:128], psf, start=True, stop=True)
                fn1 = wpool.tile([128, 128], f32, tag="fn1")
                inorm(fn1, psf, 6, 7, "c")
                m1 = wpool.tile([128, 128], f32, tag="m1")
                nc.vector.scalar_tensor_tensor(m1, an, cbc[:, 29:30], fn1,
                                               alu.mult, alu.add)
                nc.vector.tensor_scalar_add(m1, m1, cbc[:, 30:31])
                m1g = wpool.tile([128, 128], f32, tag="m1g")
                nc.scalar.activation(m1g, m1, AF.Gelu)
                vm2 = fwd_mix(m1g, 1)
                psf2 = psb.tile([128, 128], f32, tag="B")
                inv32(vm2[:, 0:64], vm2[:, 64:128], psf2, start=True, stop=True)
                fn2 = wpool.tile([128, 128], f32, tag="fn2")
                inorm(fn2, psf2, 8, 9, "d")
                m2 = wpool.tile([128, 128], f32, tag="m2")
                nc.vector.scalar_tensor_tensor(m2, m1g, cbc[:, 31:32], fn2,
                                               alu.mult, alu.add)
                nc.vector.tensor_scalar_add(m2, m2, cbc[:, 32:33])
                oimg = wpool.tile([128, 128], f32, tag="oimg")
                inorm(oimg, m2, 10, 11, "e")
                nc.vector.tensor_tensor(oimg, oimg, attn_img, alu.add)
                nc.sync.dma_start(out[tl_].rearrange("(h w) -> h w", h=128), oimg)

    nc.compile()
    return nc


# ---------------------------------------------------------------------------
# kernel()
# ---------------------------------------------------------------------------

def kernel(x, wK, wKs, bKs, wQ, wQs, bQs, wV, wVs, bVs, wP, wPs, bPs,
           wM0, wM0s, bM0s, wM1, wM1s, bM1s, norm_g, norm_b):
    global _NC, LAST_EXEC_NS
    import concourse.bass_utils as bass_utils

    loc = dict(x=x, wK=wK, wKs=wKs, bKs=bKs, wQ=wQ, wQs=wQs, bQs=bQs,
               wV=wV, wVs=wVs, bVs=bVs, wP=wP, wPs=wPs, bPs=bPs,
               wM0=wM0, wM0s=wM0s, bM0s=bM0s, wM1=wM1, wM1s=wM1s, bM1s=bM1s,
               norm_g=norm_g, norm_b=norm_b)
    args = {k: np.asarray(v, np.float32) for k, v in loc.items()}
    C = _make_consts(args)
    x64 = args["x"].reshape(64, 16384)

    if _NC is None:
        _NC = _build_nc()

    wVs_l, wPs_l, bVs_l = C["_W"]
    in_maps = []
    for c in range(8):
        hs = slice(4*c, 4*c+4)
        crow = C["_crow_base"].copy()
        crow[0, 12:16] = wVs_l[hs]
        crow[0, 16:20] = wPs_l[hs]
        crow[0, 20:24] = wPs_l[hs]*wVs_l[hs]
        crow[0, 24:28] = bVs_l[hs]
        dcwm = np.zeros((3, 16), np.float32)
        for i, o in enumerate(range(4*c, 4*c+4)):
            dcwm[:, 4*i:4*i+4] = C["DCWM"][o]
        percore = {
            "FrT2": C["FrT2"], "G2": C["G2"], "G2m": C["G2m"],
            "IDENT": C["IDENT"], "QINV": C["QINV"], "PINV": C["PINV"],
            "PPERM": C["PPERM"], "CMIX": C["CMIX"], "ONES": C["ONES"],
            "DK": np.concatenate(
                [np.concatenate([C["DK"][o, 0], C["DK"][o, 1]], 1)
                 for o in range(4*c, 4*c+4)], 1),
            "EP": np.concatenate(
                [np.concatenate([C["EP"][o, 0], C["EP"][o, 1]], 1)
                 for o in range(4*c, 4*c+4)], 1),
            "DCWM": dcwm, "CROW": crow,
            "CVF": C["CVF"][hs].reshape(8, 2048),
            "WPF": C["WPF"][hs].reshape(8, 2048),
        }
        ct = np.concatenate(
            [np.asarray(percore[n], np.float32).ravel() for n, _ in CT_TABLES])
        import ml_dtypes
        in_maps.append({
            "xs": np.ascontiguousarray(
                x64[8*c:8*c+8].astype(ml_dtypes.bfloat16)),
            "CT": np.ascontiguousarray(
                ct.reshape(1, CT_TOT).astype(ml_dtypes.bfloat16)),
        })
    t0 = time.time()
    res = bass_utils.run_bass_kernel_spmd(_NC, in_maps, core_ids=list(range(8)))
    t1 = time.time()
    LAST_EXEC_NS = (res.exec_time_ns if res.exec_time_ns
                    else int((t1 - t0) * 1e9))
    outp = np.concatenate([np.asarray(r["out"]) for r in res.results], axis=0)
    return np.ascontiguousarray(outp.reshape(2, 32, 128, 128).astype(np.float32))
